# revision 7
# baseline (speedup 1.0000x reference)
"""Multi-head attention (B=2, S=2048, D=1024, H=16) on 8 NeuronCores.

Sharding: core c handles batch b = c//4 and head-group g = c%4 (4 heads,
F = 256 features). Data-parallel over B, tensor-parallel over heads:
Wq/Wk/Wv column-sliced, Wo row-sliced; host sums the 8 partial outputs.

Device kernel (per core), everything transposed so no on-chip transposes:
  phase 1: qT = (Wq_s)^T xT, kT likewise (feature-major), v natural (s-major)
           stored with a ones-column per head (softmax denominator trick)
  phase 2: scores^T tiles (sk, sq) via paired K=64 matmuls (tile_position),
           E = exp(s/8) * keep (ACT exp + DVE mask-mult),
           U' = [v,1]^T @ E accumulated over sk -> rows 0..63 = ctx^T,
           row 64 = softmax denominator. Head pairs interleaved in the sk
           loop for PE density. Division is DEFERRED: denominators are
           collected per head, one batched ln/exp(-x) pass at the end
           (avoids ACT table thrashing), then ctxT *= 1/denom in place.
  phase 3: out_partial = ctx @ Wo_s (natural orientation), DMA out.

All matmuls run in float32r (tf32-class precision, 1 cycle/row).
ATT_DT switches the attention-weight/value dtype (f32r | bf16).
"""

import os

import numpy as np
import ml_dtypes

import concourse.tile as tile
from concourse import bacc, mybir
from concourse.bass_utils import run_bass_kernel_spmd

B, S, D, H = 2, 2048, 1024, 16
DH = D // H  # 64
NCORES = 8
GROUPS = 4  # head groups (cores per batch)
HL = H // GROUPS  # 4 heads per core
F = HL * DH  # 256 local features
SQC = 512  # sq chunk width
NSQ = S // SQC  # 4
SKT = S // 128  # 16 sk tiles
PD = D // 128  # 8 contraction chunks

FP32 = mybir.dt.float32
FP32R = mybir.dt.float32r
BF16 = mybir.dt.bfloat16

ATT_DT = FP32R if os.environ.get("ATT_DT", "f32r") == "f32r" else BF16
CW = 256 if ATT_DT == FP32R else 512  # phase-1 s-chunk width (SBUF budget)
NPC = S // CW

_CACHE = {}


def _build():
    nc = bacc.Bacc("TRN2", target_bir_lowering=False, debug=False)

    xq_d = nc.dram_tensor("xqT", [D, S], FP32R, kind="ExternalInput").ap()
    xk_d = nc.dram_tensor("xkT", [D, S], FP32R, kind="ExternalInput").ap()
    xv_d = nc.dram_tensor("xvT", [D, S], FP32R, kind="ExternalInput").ap()
    keep_d = nc.dram_tensor("keepT", [S, S], BF16, kind="ExternalInput").ap()
    wq_d = nc.dram_tensor("Wq", [D, F], FP32R, kind="ExternalInput").ap()
    wk_d = nc.dram_tensor("Wk", [D, F], FP32R, kind="ExternalInput").ap()
    wv_d = nc.dram_tensor("Wv", [D, F], FP32R, kind="ExternalInput").ap()
    wo_d = nc.dram_tensor("Wo", [F, D], FP32R, kind="ExternalInput").ap()
    bq_d = nc.dram_tensor("bq", [F, 1], FP32, kind="ExternalInput").ap()
    bk_d = nc.dram_tensor("bk", [F, 1], FP32, kind="ExternalInput").ap()
    bv_d = nc.dram_tensor("bv", [1, F], FP32, kind="ExternalInput").ap()
    out_d = nc.dram_tensor("out", [S, D], FP32, kind="ExternalOutput").ap()

    Exp = mybir.ActivationFunctionType.Exp
    Ln = mybir.ActivationFunctionType.Ln

    with tile.TileContext(nc) as tc:
        with tc.tile_pool(name="persist", bufs=1) as pp:
            qT = pp.tile([128, 2, S], FP32R, tag="qT")  # 2 f-chunks (=head pairs)
            kT = pp.tile([128, 2, S], FP32R, tag="kT")
            v2 = pp.tile([128, SKT, HL, DH + 1], ATT_DT, tag="v2")
            ctxT = pp.tile([128, 2, S], FP32R, tag="ctxT")
            wo = pp.tile([128, 2, D], FP32R, tag="wo")
            bq_sb = pp.tile([128, 2, 1], FP32, tag="bq")
            bk_sb = pp.tile([128, 2, 1], FP32, tag="bk")
            bv_bc = pp.tile([128, F], FP32, tag="bvbc")
            sums = [pp.tile([1, S], FP32, tag=f"sums{h}", name=f"sums{h}") for h in range(HL)]

            nc.sync.dma_start(out=wo[:], in_=wo_d.rearrange("(c p) n -> p c n", p=128))
            nc.sync.dma_start(out=bq_sb[:], in_=bq_d.rearrange("(c p) o -> p c o", p=128))
            nc.sync.dma_start(out=bk_sb[:], in_=bk_d.rearrange("(c p) o -> p c o", p=128))
            bv_row = pp.tile([1, F], FP32, tag="bvrow")
            nc.sync.dma_start(out=bv_row[:], in_=bv_d)
            nc.gpsimd.partition_broadcast(bv_bc[:], bv_row[:])
            ones_ap = v2[:, :, :, DH:DH + 1]
            if ATT_DT == FP32R:
                ones_ap = ones_ap.bitcast(FP32)
            nc.vector.memset(ones_ap, 1.0)

            # ---- phase 1: projections ----
            with tc.tile_pool(name="p1", bufs=2) as p1, \
                 tc.tile_pool(name="p1w", bufs=1) as p1w:
                wq = p1w.tile([128, PD, F], FP32R, tag="wq")
                wk = p1w.tile([128, PD, F], FP32R, tag="wk")
                wv = p1w.tile([128, PD, F], FP32R, tag="wv")
                nc.sync.dma_start(out=wq[:], in_=wq_d.rearrange("(c p) f -> p c f", p=128))
                nc.sync.dma_start(out=wk[:], in_=wk_d.rearrange("(c p) f -> p c f", p=128))
                nc.sync.dma_start(out=wv[:], in_=wv_d.rearrange("(c p) f -> p c f", p=128))

                with tc.tile_pool(name="psum1", bufs=2, space="PSUM") as ps1:
                    for s4 in range(NPC):
                        sl = slice(s4 * CW, (s4 + 1) * CW)
                        xq_sl = p1.tile([128, PD, CW], FP32R, tag="xq")
                        xk_sl = p1.tile([128, PD, CW], FP32R, tag="xk")
                        xv_sl = p1.tile([128, PD, CW], FP32R, tag="xv")
                        nc.sync.dma_start(out=xq_sl[:], in_=xq_d.rearrange("(c p) s -> p c s", p=128)[:, :, sl])
                        nc.sync.dma_start(out=xk_sl[:], in_=xk_d.rearrange("(c p) s -> p c s", p=128)[:, :, sl])
                        nc.sync.dma_start(out=xv_sl[:], in_=xv_d.rearrange("(c p) s -> p c s", p=128)[:, :, sl])

                        for fc in range(2):
                            fsl = slice(fc * 128, (fc + 1) * 128)
                            q_ps = ps1.tile([128, CW], FP32, tag="q_ps")
                            for d in range(PD):
                                nc.tensor.matmul(q_ps[:], wq[:, d, fsl], xq_sl[:, d, :],
                                                 start=(d == 0), stop=(d == PD - 1))
                            nc.scalar.add(qT[:, fc, sl], q_ps[:], bq_sb[:, fc, :])

                            k_ps = ps1.tile([128, CW], FP32, tag="k_ps")
                            for d in range(PD):
                                nc.tensor.matmul(k_ps[:], wk[:, d, fsl], xk_sl[:, d, :],
                                                 start=(d == 0), stop=(d == PD - 1))
                            nc.scalar.add(kT[:, fc, sl], k_ps[:], bk_sb[:, fc, :])

                        for m in range(CW // 128):  # s-subtiles of 128
                            ti = s4 * (CW // 128) + m
                            msl = slice(m * 128, (m + 1) * 128)
                            v_ps = ps1.tile([128, F], FP32, tag="v_ps")
                            for d in range(PD):
                                nc.tensor.matmul(v_ps[:], xv_sl[:, d, msl], wv[:, d, :],
                                                 start=(d == 0), stop=(d == PD - 1))
                            for hh in range(HL):
                                hsl = slice(hh * DH, (hh + 1) * DH)
                                nc.vector.tensor_add(
                                    v2[:, ti, hh, 0:DH],
                                    v_ps[:, hsl],
                                    bv_bc[:, hsl],
                                )

            # ---- phase 2: attention ----
            with tc.tile_pool(name="p2", bufs=2) as p2, \
                 tc.tile_pool(name="p2e", bufs=4) as p2e, \
                 tc.tile_pool(name="psum_st", bufs=2, space="PSUM") as ps_st, \
                 tc.tile_pool(name="psum_u", bufs=2, space="PSUM") as ps_u:
                for sq in range(NSQ):
                    qsl = slice(sq * SQC, (sq + 1) * SQC)
                    keep = p2.tile([128, SKT, SQC], BF16, tag="keep")
                    nc.sync.dma_start(
                        out=keep[:],
                        in_=keep_d.rearrange("(t p) q -> p t q", p=128)[:, :, qsl],
                    )
                    u = [ps_u.tile([128, 2, SQC], FP32, tag="u", name=f"u_sq{sq}_{i}") for i in range(2)]
                    for sk in range(SKT):
                        ksl = slice(sk * 128, (sk + 1) * 128)
                        for hp in range(2):
                            st_ps = ps_st.tile([128, 2, SQC], FP32, tag="st")
                            nc.tensor.matmul(st_ps[:, 0, :], kT[0:64, hp, ksl],
                                             qT[0:64, hp, qsl], start=True, stop=True,
                                             tile_position=(0, 0))
                            nc.tensor.matmul(st_ps[:, 1, :], kT[64:128, hp, ksl],
                                             qT[64:128, hp, qsl], start=True, stop=True,
                                             tile_position=(64, 0))
                            e_sb = p2e.tile([128, 2, SQC], ATT_DT, tag="e")
                            nc.scalar.activation(e_sb[:], st_ps[:], Exp, scale=0.125)
                            e2 = p2e.tile([128, 2, SQC], ATT_DT, tag="e2")
                            nc.vector.tensor_mul(e2[:, 0, :], e_sb[:, 0, :], keep[:, sk, :])
                            nc.vector.tensor_mul(e2[:, 1, :], e_sb[:, 1, :], keep[:, sk, :])
                            for j in range(2):
                                nc.tensor.matmul(
                                    u[hp][0:DH + 1, j, :],
                                    v2[:, sk, 2 * hp + j, :],
                                    e2[:, j, :],
                                    start=(sk == 0), stop=(sk == SKT - 1),
                                )
                    for hp in range(2):
                        for j in range(2):
                            nc.vector.tensor_copy(sums[2 * hp + j][0:1, qsl],
                                                  u[hp][DH:DH + 1, j, :])
                            nc.vector.tensor_copy(ctxT[j * DH:(j + 1) * DH, hp, qsl],
                                                  u[hp][0:DH, j, :])

            # batched reciprocal: r = exp(-ln(sum)); 2 table loads total
            with tc.tile_pool(name="rec", bufs=2) as rec:
                lns = [rec.tile([1, S], FP32, tag=f"lns{h}", name=f"lns{h}", bufs=1) for h in range(HL)]
                with tc.tile_critical():
                    for h in range(HL):
                        nc.scalar.activation(lns[h][:], sums[h][:], Ln)
                    for h in range(HL):
                        nc.scalar.activation(sums[h][:], lns[h][:], Exp, scale=-1.0)
                for h in range(HL):
                    hp, j = h // 2, h % 2
                    rb = rec.tile([128, S], FP32, tag="rb")
                    nc.gpsimd.partition_broadcast(rb[:], sums[h][:])
                    nc.vector.tensor_mul(ctxT[j * DH:(j + 1) * DH, hp, :],
                                         ctxT[j * DH:(j + 1) * DH, hp, :],
                                         rb[j * DH:(j + 1) * DH, :])

            # ---- phase 3: output projection ----
            with tc.tile_pool(name="p3", bufs=4) as p3, \
                 tc.tile_pool(name="psum3", bufs=4, space="PSUM") as ps3:
                for ti in range(S // 128):
                    tsl = slice(ti * 128, (ti + 1) * 128)
                    for n in range(2):
                        nsl = slice(n * 512, (n + 1) * 512)
                        o_ps = ps3.tile([128, 512], FP32, tag="o_ps")
                        for fc in range(2):
                            nc.tensor.matmul(o_ps[:], ctxT[:, fc, tsl], wo[:, fc, nsl],
                                             start=(fc == 0), stop=(fc == 1))
                        o_sb = p3.tile([128, 512], FP32, tag="o_sb")
                        if (ti * 2 + n) % 2 == 0:
                            nc.vector.tensor_copy(o_sb[:], o_ps[:])
                        else:
                            nc.scalar.copy(o_sb[:], o_ps[:])
                        nc.sync.dma_start(out=out_d[tsl, nsl], in_=o_sb[:])

    nc.compile()
    return nc


def kernel(query, key, value, mask, Wq, bq, Wk, bk, Wv, bv, Wo, bo, **_):
    if "nc" not in _CACHE:
        _CACHE["nc"] = _build()
    nc = _CACHE["nc"]

    query = np.asarray(query, np.float32)
    key = np.asarray(key, np.float32)
    value = np.asarray(value, np.float32)
    mask = np.asarray(mask)
    Wq = np.asarray(Wq, np.float32)
    Wk = np.asarray(Wk, np.float32)
    Wv = np.asarray(Wv, np.float32)
    Wo = np.asarray(Wo, np.float32)
    bq = np.asarray(bq, np.float32)
    bk = np.asarray(bk, np.float32)
    bv = np.asarray(bv, np.float32)
    bo = np.asarray(bo, np.float32)

    xT = {}
    keepT = {}
    for b in range(B):
        xT[b] = (
            np.ascontiguousarray(query[b].T),
            np.ascontiguousarray(key[b].T),
            np.ascontiguousarray(value[b].T),
        )
        keepT[b] = np.ascontiguousarray((~mask[b]).T).astype(ml_dtypes.bfloat16)

    wsl = {}
    for g in range(GROUPS):
        fs = slice(g * F, (g + 1) * F)
        wsl[g] = (
            np.ascontiguousarray(Wq[:, fs]),
            np.ascontiguousarray(Wk[:, fs]),
            np.ascontiguousarray(Wv[:, fs]),
            np.ascontiguousarray(Wo[fs, :]),
            np.ascontiguousarray(bq[fs].reshape(F, 1)),
            np.ascontiguousarray(bk[fs].reshape(F, 1)),
            np.ascontiguousarray(bv[fs].reshape(1, F)),
        )

    in_maps = []
    for c in range(NCORES):
        b, g = c // GROUPS, c % GROUPS
        wq_s, wk_s, wv_s, wo_s, bq_s, bk_s, bv_s = wsl[g]
        in_maps.append({
            "xqT": xT[b][0], "xkT": xT[b][1], "xvT": xT[b][2],
            "keepT": keepT[b],
            "Wq": wq_s, "Wk": wk_s, "Wv": wv_s, "Wo": wo_s,
            "bq": bq_s, "bk": bk_s, "bv": bv_s,
        })

    res = run_bass_kernel_spmd(nc, in_maps, core_ids=list(range(NCORES)))
    outs = [r["out"] for r in res.results]
    full = np.empty((B, S, D), np.float32)
    for b in range(B):
        acc = outs[GROUPS * b].astype(np.float32)
        for g in range(1, GROUPS):
            acc = acc + outs[GROUPS * b + g]
        full[b] = acc + bo
    return full


# revision 9
# speedup vs baseline: 1.3652x; 1.3652x over previous
"""Multi-head attention (B=2, S=2048, D=1024, H=16) on 8 NeuronCores.

Sharding: core c handles batch b = c//4 and head-group g = c%4 (4 heads,
F = 256 features). Data-parallel over B, tensor-parallel over heads:
Wq/Wk/Wv column-sliced, Wo row-sliced; host sums the 8 partial outputs.

Device kernel (per core), everything transposed so no on-chip transposes:
  phase 1: qT = (Wq_s)^T xT, kT likewise (feature-major), v natural (s-major)
           stored with a ones-column per head (softmax denominator trick)
  phase 2: scores^T tiles (sk, sq) via paired K=64 matmuls (tile_position),
           E = exp(s/8) * keep (ACT exp + DVE mask-mult),
           U' = [v,1]^T @ E accumulated over sk -> rows 0..63 = ctx^T,
           row 64 = softmax denominator. Head pairs interleaved in the sk
           loop for PE density. Division is DEFERRED: denominators are
           collected per head, one batched ln/exp(-x) pass at the end
           (avoids ACT table thrashing), then ctxT *= 1/denom in place.
  phase 3: out_partial = ctx @ Wo_s (natural orientation), DMA out.

All matmuls run in float32r (tf32-class precision, 1 cycle/row).
ATT_DT switches the attention-weight/value dtype (f32r | bf16).
"""

import os

import numpy as np
import ml_dtypes

import concourse.tile as tile
from concourse import bacc, mybir
from concourse.bass_utils import run_bass_kernel_spmd

B, S, D, H = 2, 2048, 1024, 16
DH = D // H  # 64
NCORES = 8
GROUPS = 4  # head groups (cores per batch)
HL = H // GROUPS  # 4 heads per core
F = HL * DH  # 256 local features
SQC = 512  # sq chunk width
NSQ = S // SQC  # 4
SKT = S // 128  # 16 sk tiles
PD = D // 128  # 8 contraction chunks

FP32 = mybir.dt.float32
FP32R = mybir.dt.float32r
BF16 = mybir.dt.bfloat16
FP16 = mybir.dt.float16

ATT_DT = {"f32r": FP32R, "fp16": FP16, "bf16": BF16}[os.environ.get("ATT_DT", "fp16")]
CW = 256  # phase-1 s-chunk width (SBUF budget: x slices stay fp32r)
NPC = S // CW

_CACHE = {}


def _build():
    nc = bacc.Bacc("TRN2", target_bir_lowering=False, debug=False)

    xq_d = nc.dram_tensor("xqT", [D, S], FP32R, kind="ExternalInput").ap()
    xk_d = nc.dram_tensor("xkT", [D, S], FP32R, kind="ExternalInput").ap()
    xv_d = nc.dram_tensor("xvT", [D, S], FP32R, kind="ExternalInput").ap()
    KEEP_DT = FP16 if ATT_DT != BF16 else BF16
    keep_d = nc.dram_tensor("keepT", [S, S], KEEP_DT, kind="ExternalInput").ap()
    wq_d = nc.dram_tensor("Wq", [D, F], FP32R, kind="ExternalInput").ap()
    wk_d = nc.dram_tensor("Wk", [D, F], FP32R, kind="ExternalInput").ap()
    wv_d = nc.dram_tensor("Wv", [D, F], FP32R, kind="ExternalInput").ap()
    wo_d = nc.dram_tensor("Wo", [F, D], FP32R, kind="ExternalInput").ap()
    bq_d = nc.dram_tensor("bq", [F, 1], FP32, kind="ExternalInput").ap()
    bk_d = nc.dram_tensor("bk", [F, 1], FP32, kind="ExternalInput").ap()
    bv_d = nc.dram_tensor("bv", [1, F], FP32, kind="ExternalInput").ap()
    out_d = nc.dram_tensor("out", [S, D], FP32, kind="ExternalOutput").ap()

    Exp = mybir.ActivationFunctionType.Exp
    Ln = mybir.ActivationFunctionType.Ln

    with tile.TileContext(nc) as tc:
        with tc.tile_pool(name="persist", bufs=1) as pp:
            qT = pp.tile([128, 2, S], FP32R, tag="qT")  # 2 f-chunks (=head pairs)
            kT = pp.tile([128, 2, S], FP32R, tag="kT")
            v2 = pp.tile([128, SKT, HL, DH + 1], ATT_DT, tag="v2")
            ctxT = pp.tile([128, 2, S], FP32R, tag="ctxT")
            wo = pp.tile([128, 2, D], FP32R, tag="wo")
            bq_sb = pp.tile([128, 2, 1], FP32, tag="bq")
            bk_sb = pp.tile([128, 2, 1], FP32, tag="bk")
            bv_bc = pp.tile([128, F], FP32, tag="bvbc")
            sums = [pp.tile([1, S], FP32, tag=f"sums{h}", name=f"sums{h}") for h in range(HL)]

            nc.sync.dma_start(out=wo[:], in_=wo_d.rearrange("(c p) n -> p c n", p=128))
            nc.sync.dma_start(out=bq_sb[:], in_=bq_d.rearrange("(c p) o -> p c o", p=128))
            nc.sync.dma_start(out=bk_sb[:], in_=bk_d.rearrange("(c p) o -> p c o", p=128))
            bv_row = pp.tile([1, F], FP32, tag="bvrow")
            nc.sync.dma_start(out=bv_row[:], in_=bv_d)
            nc.gpsimd.partition_broadcast(bv_bc[:], bv_row[:])
            ones_ap = v2[:, :, :, DH:DH + 1]
            if ATT_DT == FP32R:
                ones_ap = ones_ap.bitcast(FP32)
            nc.vector.memset(ones_ap, 1.0)

            # ---- phase 1: projections ----
            with tc.tile_pool(name="p1", bufs=2) as p1, \
                 tc.tile_pool(name="p1w", bufs=1) as p1w:
                wq = p1w.tile([128, PD, F], FP32R, tag="wq")
                wk = p1w.tile([128, PD, F], FP32R, tag="wk")
                wv = p1w.tile([128, PD, F], FP32R, tag="wv")
                nc.sync.dma_start(out=wq[:], in_=wq_d.rearrange("(c p) f -> p c f", p=128))
                nc.sync.dma_start(out=wk[:], in_=wk_d.rearrange("(c p) f -> p c f", p=128))
                nc.sync.dma_start(out=wv[:], in_=wv_d.rearrange("(c p) f -> p c f", p=128))

                with tc.tile_pool(name="psum1", bufs=2, space="PSUM") as ps1:
                    for s4 in range(NPC):
                        sl = slice(s4 * CW, (s4 + 1) * CW)
                        xq_sl = p1.tile([128, PD, CW], FP32R, tag="xq")
                        xk_sl = p1.tile([128, PD, CW], FP32R, tag="xk")
                        xv_sl = p1.tile([128, PD, CW], FP32R, tag="xv")
                        nc.sync.dma_start(out=xq_sl[:], in_=xq_d.rearrange("(c p) s -> p c s", p=128)[:, :, sl])
                        nc.sync.dma_start(out=xk_sl[:], in_=xk_d.rearrange("(c p) s -> p c s", p=128)[:, :, sl])
                        nc.sync.dma_start(out=xv_sl[:], in_=xv_d.rearrange("(c p) s -> p c s", p=128)[:, :, sl])

                        for fc in range(2):
                            fsl = slice(fc * 128, (fc + 1) * 128)
                            q_ps = ps1.tile([128, CW], FP32, tag="q_ps")
                            for d in range(PD):
                                nc.tensor.matmul(q_ps[:], wq[:, d, fsl], xq_sl[:, d, :],
                                                 start=(d == 0), stop=(d == PD - 1))
                            nc.scalar.add(qT[:, fc, sl], q_ps[:], bq_sb[:, fc, :])

                            k_ps = ps1.tile([128, CW], FP32, tag="k_ps")
                            for d in range(PD):
                                nc.tensor.matmul(k_ps[:], wk[:, d, fsl], xk_sl[:, d, :],
                                                 start=(d == 0), stop=(d == PD - 1))
                            nc.scalar.add(kT[:, fc, sl], k_ps[:], bk_sb[:, fc, :])

                        for m in range(CW // 128):  # s-subtiles of 128
                            ti = s4 * (CW // 128) + m
                            msl = slice(m * 128, (m + 1) * 128)
                            v_ps = ps1.tile([128, F], FP32, tag="v_ps")
                            for d in range(PD):
                                nc.tensor.matmul(v_ps[:], xv_sl[:, d, msl], wv[:, d, :],
                                                 start=(d == 0), stop=(d == PD - 1))
                            for hh in range(HL):
                                hsl = slice(hh * DH, (hh + 1) * DH)
                                nc.vector.tensor_add(
                                    v2[:, ti, hh, 0:DH],
                                    v_ps[:, hsl],
                                    bv_bc[:, hsl],
                                )

            # ---- phase 2: attention ----
            with tc.tile_pool(name="p2", bufs=2) as p2, \
                 tc.tile_pool(name="p2e", bufs=4) as p2e, \
                 tc.tile_pool(name="psum_st", bufs=2, space="PSUM") as ps_st, \
                 tc.tile_pool(name="psum_u", bufs=2, space="PSUM") as ps_u:
                for sq in range(NSQ):
                    qsl = slice(sq * SQC, (sq + 1) * SQC)
                    keep = p2.tile([128, SKT, SQC], KEEP_DT, tag="keep")
                    nc.gpsimd.dma_start(
                        out=keep[:],
                        in_=keep_d.rearrange("(t p) q -> p t q", p=128)[:, :, qsl],
                    )
                    for hp in range(2):
                        u_ps = ps_u.tile([128, 2, SQC], FP32, tag="u", name=f"u_sq{sq}_{hp}")
                        for sk in range(SKT):
                            ksl = slice(sk * 128, (sk + 1) * 128)
                            st_ps = ps_st.tile([128, 2, SQC], FP32, tag="st")
                            nc.tensor.matmul(st_ps[:, 0, :], kT[0:64, hp, ksl],
                                             qT[0:64, hp, qsl], start=True, stop=True,
                                             tile_position=(0, 0))
                            nc.tensor.matmul(st_ps[:, 1, :], kT[64:128, hp, ksl],
                                             qT[64:128, hp, qsl], start=True, stop=True,
                                             tile_position=(64, 0))
                            e_sb = p2e.tile([128, 2, SQC], ATT_DT, tag="e")
                            nc.scalar.activation(e_sb[:], st_ps[:], Exp, scale=0.125)
                            e2 = p2e.tile([128, 2, SQC], ATT_DT, tag="e2")
                            nc.vector.tensor_mul(e2[:, 0, :], e_sb[:, 0, :], keep[:, sk, :])
                            nc.vector.tensor_mul(e2[:, 1, :], e_sb[:, 1, :], keep[:, sk, :])
                            for j in range(2):
                                nc.tensor.matmul(
                                    u_ps[0:DH + 1, j, :],
                                    v2[:, sk, 2 * hp + j, :],
                                    e2[:, j, :],
                                    start=(sk == 0), stop=(sk == SKT - 1),
                                )
                        for j in range(2):
                            nc.vector.tensor_copy(sums[2 * hp + j][0:1, qsl],
                                                  u_ps[DH:DH + 1, j, :])
                            nc.vector.tensor_copy(ctxT[j * DH:(j + 1) * DH, hp, qsl],
                                                  u_ps[0:DH, j, :])

            # batched reciprocal: r = exp(-ln(sum)); 2 table loads total
            with tc.tile_pool(name="rec", bufs=2) as rec:
                lns = [rec.tile([1, S], FP32, tag=f"lns{h}", name=f"lns{h}", bufs=1) for h in range(HL)]
                with tc.tile_critical():
                    for h in range(HL):
                        nc.scalar.activation(lns[h][:], sums[h][:], Ln)
                    for h in range(HL):
                        nc.scalar.activation(sums[h][:], lns[h][:], Exp, scale=-1.0)
                for h in range(HL):
                    hp, j = h // 2, h % 2
                    rb = rec.tile([128, S], FP32, tag="rb")
                    nc.gpsimd.partition_broadcast(rb[:], sums[h][:])
                    nc.vector.tensor_mul(ctxT[j * DH:(j + 1) * DH, hp, :],
                                         ctxT[j * DH:(j + 1) * DH, hp, :],
                                         rb[j * DH:(j + 1) * DH, :])

            # ---- phase 3: output projection ----
            with tc.tile_pool(name="p3", bufs=4) as p3, \
                 tc.tile_pool(name="psum3", bufs=4, space="PSUM") as ps3:
                for ti in range(S // 128):
                    tsl = slice(ti * 128, (ti + 1) * 128)
                    for n in range(2):
                        nsl = slice(n * 512, (n + 1) * 512)
                        o_ps = ps3.tile([128, 512], FP32, tag="o_ps")
                        for fc in range(2):
                            nc.tensor.matmul(o_ps[:], ctxT[:, fc, tsl], wo[:, fc, nsl],
                                             start=(fc == 0), stop=(fc == 1))
                        o_sb = p3.tile([128, 512], FP32, tag="o_sb")
                        if (ti * 2 + n) % 2 == 0:
                            nc.vector.tensor_copy(o_sb[:], o_ps[:])
                        else:
                            nc.scalar.copy(o_sb[:], o_ps[:])
                        nc.gpsimd.dma_start(out=out_d[tsl, nsl], in_=o_sb[:])

    nc.compile()
    return nc


def kernel(query, key, value, mask, Wq, bq, Wk, bk, Wv, bv, Wo, bo, **_):
    if "nc" not in _CACHE:
        _CACHE["nc"] = _build()
    nc = _CACHE["nc"]

    query = np.asarray(query, np.float32)
    key = np.asarray(key, np.float32)
    value = np.asarray(value, np.float32)
    mask = np.asarray(mask)
    Wq = np.asarray(Wq, np.float32)
    Wk = np.asarray(Wk, np.float32)
    Wv = np.asarray(Wv, np.float32)
    Wo = np.asarray(Wo, np.float32)
    bq = np.asarray(bq, np.float32)
    bk = np.asarray(bk, np.float32)
    bv = np.asarray(bv, np.float32)
    bo = np.asarray(bo, np.float32)

    xT = {}
    keepT = {}
    for b in range(B):
        xT[b] = (
            np.ascontiguousarray(query[b].T),
            np.ascontiguousarray(key[b].T),
            np.ascontiguousarray(value[b].T),
        )
        keepT[b] = np.ascontiguousarray((~mask[b]).T).astype(
            ml_dtypes.bfloat16 if os.environ.get("ATT_DT") == "bf16" else np.float16)

    wsl = {}
    for g in range(GROUPS):
        fs = slice(g * F, (g + 1) * F)
        wsl[g] = (
            np.ascontiguousarray(Wq[:, fs]),
            np.ascontiguousarray(Wk[:, fs]),
            np.ascontiguousarray(Wv[:, fs]),
            np.ascontiguousarray(Wo[fs, :]),
            np.ascontiguousarray(bq[fs].reshape(F, 1)),
            np.ascontiguousarray(bk[fs].reshape(F, 1)),
            np.ascontiguousarray(bv[fs].reshape(1, F)),
        )

    in_maps = []
    for c in range(NCORES):
        b, g = c // GROUPS, c % GROUPS
        wq_s, wk_s, wv_s, wo_s, bq_s, bk_s, bv_s = wsl[g]
        in_maps.append({
            "xqT": xT[b][0], "xkT": xT[b][1], "xvT": xT[b][2],
            "keepT": keepT[b],
            "Wq": wq_s, "Wk": wk_s, "Wv": wv_s, "Wo": wo_s,
            "bq": bq_s, "bk": bk_s, "bv": bv_s,
        })

    res = run_bass_kernel_spmd(nc, in_maps, core_ids=list(range(NCORES)))
    outs = [r["out"] for r in res.results]
    full = np.empty((B, S, D), np.float32)
    for b in range(B):
        acc = outs[GROUPS * b].astype(np.float32)
        for g in range(1, GROUPS):
            acc = acc + outs[GROUPS * b + g]
        full[b] = acc + bo
    return full


# revision 10
# speedup vs baseline: 1.5069x; 1.1038x over previous
"""Multi-head attention (B=2, S=2048, D=1024, H=16) on 8 NeuronCores.

Sharding: core c handles batch b = c//4 and head-group g = c%4 (4 heads,
F = 256 features). Data-parallel over B, tensor-parallel over heads:
Wq/Wk/Wv column-sliced, Wo row-sliced; host sums the 8 partial outputs.

Device kernel (per core), everything transposed so no on-chip transposes:
  phase 1: qT = (Wq_s)^T xT, kT likewise (feature-major), v natural (s-major)
           stored with a ones-column per head (softmax denominator trick)
  phase 2: scores^T tiles (sk, sq) via paired K=64 matmuls (tile_position),
           E = exp(s/8) * keep (ACT exp + DVE mask-mult),
           U' = [v,1]^T @ E accumulated over sk -> rows 0..63 = ctx^T,
           row 64 = softmax denominator. Head pairs interleaved in the sk
           loop for PE density. Division is DEFERRED: denominators are
           collected per head, one batched ln/exp(-x) pass at the end
           (avoids ACT table thrashing), then ctxT *= 1/denom in place.
  phase 3: out_partial = ctx @ Wo_s (natural orientation), DMA out.

All matmuls run in float32r (tf32-class precision, 1 cycle/row).
ATT_DT switches the attention-weight/value dtype (f32r | bf16).
"""

import os

import numpy as np
import ml_dtypes

import concourse.tile as tile
from concourse import bacc, mybir
from concourse.bass_utils import run_bass_kernel_spmd

B, S, D, H = 2, 2048, 1024, 16
DH = D // H  # 64
NCORES = 8
GROUPS = 4  # head groups (cores per batch)
HL = H // GROUPS  # 4 heads per core
F = HL * DH  # 256 local features
SQC = 512  # sq chunk width
NSQ = S // SQC  # 4
SKT = S // 128  # 16 sk tiles
PD = D // 128  # 8 contraction chunks

FP32 = mybir.dt.float32
FP32R = mybir.dt.float32r
BF16 = mybir.dt.bfloat16
FP16 = mybir.dt.float16

ATT_DT = {"f32r": FP32R, "fp16": FP16, "bf16": BF16}[os.environ.get("ATT_DT", "fp16")]
CW = 256  # phase-1 s-chunk width (SBUF budget: x slices stay fp32r)
NPC = S // CW

_CACHE = {}


def _build():
    nc = bacc.Bacc("TRN2", target_bir_lowering=False, debug=False)

    xq_d = nc.dram_tensor("xqT", [D, S], FP32R, kind="ExternalInput").ap()
    xk_d = nc.dram_tensor("xkT", [D, S], FP32R, kind="ExternalInput").ap()
    xv_d = nc.dram_tensor("xvT", [D, S], FP32R, kind="ExternalInput").ap()
    KEEP_DT = FP16 if ATT_DT != BF16 else BF16
    keep_d = nc.dram_tensor("keepT", [S, S], KEEP_DT, kind="ExternalInput").ap()
    wq_d = nc.dram_tensor("Wq", [D, F], FP32R, kind="ExternalInput").ap()
    wk_d = nc.dram_tensor("Wk", [D, F], FP32R, kind="ExternalInput").ap()
    wv_d = nc.dram_tensor("Wv", [D, F], FP32R, kind="ExternalInput").ap()
    wo_d = nc.dram_tensor("Wo", [F, D], FP16, kind="ExternalInput").ap()
    bq_d = nc.dram_tensor("bq", [F, 1], FP32, kind="ExternalInput").ap()
    bk_d = nc.dram_tensor("bk", [F, 1], FP32, kind="ExternalInput").ap()
    bv_d = nc.dram_tensor("bv", [1, F], FP32, kind="ExternalInput").ap()
    out_d = nc.dram_tensor("out", [S, D], FP32, kind="ExternalOutput").ap()

    Exp = mybir.ActivationFunctionType.Exp
    Ln = mybir.ActivationFunctionType.Ln

    with tile.TileContext(nc) as tc:
        with tc.tile_pool(name="persist", bufs=1) as pp:
            qT = pp.tile([128, 2, S], FP32R, tag="qT")  # 2 f-chunks (=head pairs)
            kT = pp.tile([128, 2, S], FP32R, tag="kT")
            v2 = pp.tile([128, SKT, HL, DH + 1], ATT_DT, tag="v2")
            ctxT = pp.tile([128, 2, S], FP16, tag="ctxT")
            wo = pp.tile([128, 2, D], FP16, tag="wo")
            bq_sb = pp.tile([128, 2, 1], FP32, tag="bq")
            bk_sb = pp.tile([128, 2, 1], FP32, tag="bk")
            bv_bc = pp.tile([128, F], FP32, tag="bvbc")
            sums = [pp.tile([1, S], FP32, tag=f"sums{h}", name=f"sums{h}") for h in range(HL)]

            nc.sync.dma_start(out=wo[:], in_=wo_d.rearrange("(c p) n -> p c n", p=128))
            nc.sync.dma_start(out=bq_sb[:], in_=bq_d.rearrange("(c p) o -> p c o", p=128))
            nc.sync.dma_start(out=bk_sb[:], in_=bk_d.rearrange("(c p) o -> p c o", p=128))
            wu = pp.tile([128, 64], FP16, tag="wu")
            bv_row = pp.tile([1, F], FP32, tag="bvrow")
            nc.sync.dma_start(out=bv_row[:], in_=bv_d)
            nc.gpsimd.partition_broadcast(bv_bc[:], bv_row[:])
            nc.vector.memset(wu[:], 0.125)
            ones_ap = v2[:, :, :, DH:DH + 1]
            if ATT_DT == FP32R:
                ones_ap = ones_ap.bitcast(FP32)
            nc.vector.memset(ones_ap, 1.0)

            # ---- phase 1: projections ----
            with tc.tile_pool(name="p1", bufs=2) as p1, \
                 tc.tile_pool(name="p1w", bufs=1) as p1w:
                wq = p1w.tile([128, PD, F], FP32R, tag="wq")
                wk = p1w.tile([128, PD, F], FP32R, tag="wk")
                wv = p1w.tile([128, PD, F], FP32R, tag="wv")
                nc.sync.dma_start(out=wq[:], in_=wq_d.rearrange("(c p) f -> p c f", p=128))
                nc.sync.dma_start(out=wk[:], in_=wk_d.rearrange("(c p) f -> p c f", p=128))
                nc.sync.dma_start(out=wv[:], in_=wv_d.rearrange("(c p) f -> p c f", p=128))

                with tc.tile_pool(name="psum1", bufs=2, space="PSUM") as ps1:
                    for s4 in range(NPC):
                        sl = slice(s4 * CW, (s4 + 1) * CW)
                        xq_sl = p1.tile([128, PD, CW], FP32R, tag="xq")
                        xk_sl = p1.tile([128, PD, CW], FP32R, tag="xk")
                        xv_sl = p1.tile([128, PD, CW], FP32R, tag="xv")
                        nc.sync.dma_start(out=xq_sl[:], in_=xq_d.rearrange("(c p) s -> p c s", p=128)[:, :, sl])
                        nc.scalar.dma_start(out=xk_sl[:], in_=xk_d.rearrange("(c p) s -> p c s", p=128)[:, :, sl])
                        nc.gpsimd.dma_start(out=xv_sl[:], in_=xv_d.rearrange("(c p) s -> p c s", p=128)[:, :, sl])

                        for fc in range(2):
                            fsl = slice(fc * 128, (fc + 1) * 128)
                            q_ps = ps1.tile([128, CW], FP32, tag="q_ps")
                            for d in range(PD):
                                nc.tensor.matmul(q_ps[:], wq[:, d, fsl], xq_sl[:, d, :],
                                                 start=(d == 0), stop=(d == PD - 1))
                            nc.scalar.add(qT[:, fc, sl], q_ps[:], bq_sb[:, fc, :])

                            k_ps = ps1.tile([128, CW], FP32, tag="k_ps")
                            for d in range(PD):
                                nc.tensor.matmul(k_ps[:], wk[:, d, fsl], xk_sl[:, d, :],
                                                 start=(d == 0), stop=(d == PD - 1))
                            nc.scalar.add(kT[:, fc, sl], k_ps[:], bk_sb[:, fc, :])

                        for m in range(CW // 128):  # s-subtiles of 128
                            ti = s4 * (CW // 128) + m
                            msl = slice(m * 128, (m + 1) * 128)
                            v_ps = ps1.tile([128, F], FP32, tag="v_ps")
                            for d in range(PD):
                                nc.tensor.matmul(v_ps[:], xv_sl[:, d, msl], wv[:, d, :],
                                                 start=(d == 0), stop=(d == PD - 1))
                            for hh in range(HL):
                                hsl = slice(hh * DH, (hh + 1) * DH)
                                nc.vector.tensor_add(
                                    v2[:, ti, hh, 0:DH],
                                    v_ps[:, hsl],
                                    bv_bc[:, hsl],
                                )

            # ---- phase 2: attention ----
            with tc.tile_pool(name="p2", bufs=2) as p2, \
                 tc.tile_pool(name="p2e", bufs=4) as p2e, \
                 tc.tile_pool(name="psum_st", bufs=3, space="PSUM") as ps_st, \
                 tc.tile_pool(name="psum_u", bufs=1, space="PSUM") as ps_u:
                wu_ps = ps_st.tile([128, 2, SQC], FP32, tag="st", name="wu_ps")
                for i in range(40):
                    nc.tensor.matmul(wu_ps[0:64, 0, 0:64], wu[:], wu[:],
                                     start=True, stop=True)
                for sq in range(NSQ):
                    qsl = slice(sq * SQC, (sq + 1) * SQC)
                    keep = p2.tile([128, SKT, SQC], KEEP_DT, tag="keep")
                    nc.gpsimd.dma_start(
                        out=keep[:],
                        in_=keep_d.rearrange("(t p) q -> p t q", p=128)[:, :, qsl],
                    )
                    for hp in range(2):
                        u_ps = ps_u.tile([128, 2, SQC], FP32, tag="u", name=f"u_sq{sq}_{hp}")
                        for sk in range(SKT):
                            ksl = slice(sk * 128, (sk + 1) * 128)
                            st_ps = ps_st.tile([128, 2, SQC], FP32, tag="st")
                            nc.tensor.matmul(st_ps[:, 0, :], kT[0:64, hp, ksl],
                                             qT[0:64, hp, qsl], start=True, stop=True,
                                             tile_position=(0, 0))
                            nc.tensor.matmul(st_ps[:, 1, :], kT[64:128, hp, ksl],
                                             qT[64:128, hp, qsl], start=True, stop=True,
                                             tile_position=(64, 0))
                            e_sb = p2e.tile([128, 2, SQC], ATT_DT, tag="e")
                            nc.scalar.activation(e_sb[:], st_ps[:], Exp, scale=0.125)
                            e2 = p2e.tile([128, 2, SQC], ATT_DT, tag="e2")
                            nc.vector.tensor_mul(e2[:, 0, :], e_sb[:, 0, :], keep[:, sk, :])
                            nc.vector.tensor_mul(e2[:, 1, :], e_sb[:, 1, :], keep[:, sk, :])
                            for j in range(2):
                                nc.tensor.matmul(
                                    u_ps[0:DH + 1, j, :],
                                    v2[:, sk, 2 * hp + j, :],
                                    e2[:, j, :],
                                    start=(sk == 0), stop=(sk == SKT - 1),
                                )
                        for j in range(2):
                            nc.vector.tensor_copy(sums[2 * hp + j][0:1, qsl],
                                                  u_ps[DH:DH + 1, j, :])
                            nc.scalar.copy(ctxT[j * DH:(j + 1) * DH, hp, qsl],
                                           u_ps[0:DH, j, :])

            # batched reciprocal: r = exp(-ln(sum)); 2 table loads total
            with tc.tile_pool(name="rec", bufs=2) as rec:
                lns = [rec.tile([1, S], FP32, tag=f"lns{h}", name=f"lns{h}", bufs=1) for h in range(HL)]
                with tc.tile_critical():
                    for h in range(HL):
                        nc.scalar.activation(lns[h][:], sums[h][:], Ln)
                    for h in range(HL):
                        nc.scalar.activation(sums[h][:], lns[h][:], Exp, scale=-1.0)
                for h in range(HL):
                    hp, j = h // 2, h % 2
                    rb = rec.tile([128, S], FP32, tag="rb")
                    nc.gpsimd.partition_broadcast(rb[:], sums[h][:])
                    nc.vector.tensor_mul(ctxT[j * DH:(j + 1) * DH, hp, :],
                                         ctxT[j * DH:(j + 1) * DH, hp, :],
                                         rb[j * DH:(j + 1) * DH, :])

            # ---- phase 3: output projection ----
            with tc.tile_pool(name="p3", bufs=4) as p3, \
                 tc.tile_pool(name="psum3", bufs=4, space="PSUM") as ps3:
                wu3_ps = ps3.tile([128, 512], FP32, tag="o_ps", name="wu3_ps")
                for i in range(40):
                    nc.tensor.matmul(wu3_ps[0:64, 0:64], wu[:], wu[:],
                                     start=True, stop=True)
                for ti in range(S // 128):
                    tsl = slice(ti * 128, (ti + 1) * 128)
                    for n in range(2):
                        nsl = slice(n * 512, (n + 1) * 512)
                        o_ps = ps3.tile([128, 512], FP32, tag="o_ps")
                        for fc in range(2):
                            nc.tensor.matmul(o_ps[:], ctxT[:, fc, tsl], wo[:, fc, nsl],
                                             start=(fc == 0), stop=(fc == 1))
                        o_sb = p3.tile([128, 512], FP32, tag="o_sb")
                        if (ti * 2 + n) % 2 == 0:
                            nc.vector.tensor_copy(o_sb[:], o_ps[:])
                        else:
                            nc.scalar.copy(o_sb[:], o_ps[:])
                        nc.gpsimd.dma_start(out=out_d[tsl, nsl], in_=o_sb[:])

    nc.compile()
    return nc


def kernel(query, key, value, mask, Wq, bq, Wk, bk, Wv, bv, Wo, bo, **_):
    if "nc" not in _CACHE:
        _CACHE["nc"] = _build()
    nc = _CACHE["nc"]

    query = np.asarray(query, np.float32)
    key = np.asarray(key, np.float32)
    value = np.asarray(value, np.float32)
    mask = np.asarray(mask)
    Wq = np.asarray(Wq, np.float32)
    Wk = np.asarray(Wk, np.float32)
    Wv = np.asarray(Wv, np.float32)
    Wo = np.asarray(Wo, np.float32)
    bq = np.asarray(bq, np.float32)
    bk = np.asarray(bk, np.float32)
    bv = np.asarray(bv, np.float32)
    bo = np.asarray(bo, np.float32)

    xT = {}
    keepT = {}
    for b in range(B):
        xT[b] = (
            np.ascontiguousarray(query[b].T),
            np.ascontiguousarray(key[b].T),
            np.ascontiguousarray(value[b].T),
        )
        keepT[b] = np.ascontiguousarray((~mask[b]).T).astype(
            ml_dtypes.bfloat16 if os.environ.get("ATT_DT") == "bf16" else np.float16)

    wsl = {}
    for g in range(GROUPS):
        fs = slice(g * F, (g + 1) * F)
        wsl[g] = (
            np.ascontiguousarray(Wq[:, fs]),
            np.ascontiguousarray(Wk[:, fs]),
            np.ascontiguousarray(Wv[:, fs]),
            np.ascontiguousarray(Wo[fs, :]).astype(np.float16),
            np.ascontiguousarray(bq[fs].reshape(F, 1)),
            np.ascontiguousarray(bk[fs].reshape(F, 1)),
            np.ascontiguousarray(bv[fs].reshape(1, F)),
        )

    in_maps = []
    for c in range(NCORES):
        b, g = c // GROUPS, c % GROUPS
        wq_s, wk_s, wv_s, wo_s, bq_s, bk_s, bv_s = wsl[g]
        in_maps.append({
            "xqT": xT[b][0], "xkT": xT[b][1], "xvT": xT[b][2],
            "keepT": keepT[b],
            "Wq": wq_s, "Wk": wk_s, "Wv": wv_s, "Wo": wo_s,
            "bq": bq_s, "bk": bk_s, "bv": bv_s,
        })

    res = run_bass_kernel_spmd(nc, in_maps, core_ids=list(range(NCORES)))
    outs = [r["out"] for r in res.results]
    full = np.empty((B, S, D), np.float32)
    for b in range(B):
        acc = outs[GROUPS * b].astype(np.float32)
        for g in range(1, GROUPS):
            acc = acc + outs[GROUPS * b + g]
        full[b] = acc + bo
    return full


# revision 11
# speedup vs baseline: 1.5456x; 1.0257x over previous
"""Multi-head attention (B=2, S=2048, D=1024, H=16) on 8 NeuronCores.

Sharding: core c handles batch b = c//4 and head-group g = c%4 (4 heads,
F = 256 features). Data-parallel over B, tensor-parallel over heads:
Wq/Wk/Wv column-sliced, Wo row-sliced; host sums the 8 partial outputs.

Host pre-tiles every tensor into the exact SBUF layout so each DMA reads
large contiguous blocks per partition.

Device kernel (per core), everything transposed so no on-chip transposes:
  phase 1: qT/kT feature-major via fp32r matmuls, v s-major with a
           ones-column per head (softmax denominator trick), fp16.
  phase 2 (per head-pair, per sq chunk): scores^T tiles (sk, sq) via
           paired K=64 matmuls (tile_position row groups), E = exp(s/8)
           (ACT) * keep (DVE, fp16 2x mode), U' = [v,1]^T @ E accumulated
           over sk in PSUM; denominators collected per head; per-pair
           batched recip r = exp(-ln(sum)) -> ctxT = U * r (fp16).
  phase 3: out_partial = ctx @ Wo (fp16 matmuls), DMA out fp32.

Projections and scores run in float32r (tf32-class precision); the
attention-weight/value/output-projection path runs in fp16.
"""

import os

import numpy as np
import ml_dtypes

import concourse.tile as tile
from concourse import bacc, mybir
from concourse.bass_utils import run_bass_kernel_spmd

B, S, D, H = 2, 2048, 1024, 16
DH = D // H  # 64
NCORES = 8
GROUPS = 4  # head groups (cores per batch)
HL = H // GROUPS  # 4 heads per core
F = HL * DH  # 256 local features
SQC = 512  # sq chunk width
NSQ = S // SQC  # 4
SKT = S // 128  # 16 sk tiles
PD = D // 128  # 8 contraction chunks
CW = 256  # phase-1 s-chunk width
NPC = S // CW

FP32 = mybir.dt.float32
FP32R = mybir.dt.float32r
BF16 = mybir.dt.bfloat16
FP16 = mybir.dt.float16

_CACHE = {}


def _build():
    nc = bacc.Bacc("TRN2", target_bir_lowering=False, debug=False)

    xq_d = nc.dram_tensor("xqT", [NPC, 128, PD, CW], FP32R, kind="ExternalInput").ap()
    xk_d = nc.dram_tensor("xkT", [NPC, 128, PD, CW], FP32R, kind="ExternalInput").ap()
    xv_d = nc.dram_tensor("xvT", [NPC, 128, PD, CW], FP32R, kind="ExternalInput").ap()
    keep_d = nc.dram_tensor("keepT", [NSQ, 128, SKT, SQC], FP16, kind="ExternalInput").ap()
    wq_d = nc.dram_tensor("Wq", [128, PD, F], FP32R, kind="ExternalInput").ap()
    wk_d = nc.dram_tensor("Wk", [128, PD, F], FP32R, kind="ExternalInput").ap()
    wv_d = nc.dram_tensor("Wv", [128, PD, F], FP32R, kind="ExternalInput").ap()
    wo_d = nc.dram_tensor("Wo", [128, 2, D], FP16, kind="ExternalInput").ap()
    bq_d = nc.dram_tensor("bq", [F, 1], FP32, kind="ExternalInput").ap()
    bk_d = nc.dram_tensor("bk", [F, 1], FP32, kind="ExternalInput").ap()
    bv_d = nc.dram_tensor("bv", [1, F], FP32, kind="ExternalInput").ap()
    out_d = nc.dram_tensor("out", [S, D], FP32, kind="ExternalOutput").ap()

    Exp = mybir.ActivationFunctionType.Exp
    Ln = mybir.ActivationFunctionType.Ln

    with tile.TileContext(nc) as tc:
        with tc.tile_pool(name="persist", bufs=1) as pp:
            qT = pp.tile([128, 2, S], FP32R, tag="qT")  # 2 f-chunks (=head pairs)
            kT = pp.tile([128, 2, S], FP32R, tag="kT")
            v2 = pp.tile([128, SKT, HL, DH + 1], FP16, tag="v2")
            ctxT = pp.tile([128, 2, S], FP16, tag="ctxT")
            wo = pp.tile([128, 2, D], FP16, tag="wo")
            bq_sb = pp.tile([128, 2, 1], FP32, tag="bq")
            bk_sb = pp.tile([128, 2, 1], FP32, tag="bk")
            bv_bc = pp.tile([128, F], FP32, tag="bvbc")
            sums = [pp.tile([1, S], FP32, tag=f"sums{h}", name=f"sums{h}") for h in range(HL)]
            wu = pp.tile([128, 64], FP16, tag="wu")
            bv_row = pp.tile([1, F], FP32, tag="bvrow")

            nc.sync.dma_start(out=wo[:], in_=wo_d)
            nc.sync.dma_start(out=bq_sb[:], in_=bq_d.rearrange("(c p) o -> p c o", p=128))
            nc.sync.dma_start(out=bk_sb[:], in_=bk_d.rearrange("(c p) o -> p c o", p=128))
            nc.sync.dma_start(out=bv_row[:], in_=bv_d)
            nc.gpsimd.partition_broadcast(bv_bc[:], bv_row[:])
            nc.vector.memset(wu[:], 0.125)
            nc.vector.memset(v2[:, :, :, DH:DH + 1].bitcast(FP16), 1.0)

            # ---- phase 1: projections ----
            with tc.tile_pool(name="p1", bufs=2) as p1, \
                 tc.tile_pool(name="p1w", bufs=1) as p1w:
                wq = p1w.tile([128, PD, F], FP32R, tag="wq")
                wk = p1w.tile([128, PD, F], FP32R, tag="wk")
                wv = p1w.tile([128, PD, F], FP32R, tag="wv")
                nc.sync.dma_start(out=wq[:], in_=wq_d)
                nc.scalar.dma_start(out=wk[:], in_=wk_d)
                nc.gpsimd.dma_start(out=wv[:], in_=wv_d)

                with tc.tile_pool(name="psum1", bufs=2, space="PSUM") as ps1:
                    for s4 in range(NPC):
                        sl = slice(s4 * CW, (s4 + 1) * CW)
                        xq_sl = p1.tile([128, PD, CW], FP32R, tag="xq")
                        xk_sl = p1.tile([128, PD, CW], FP32R, tag="xk")
                        xv_sl = p1.tile([128, PD, CW], FP32R, tag="xv")
                        nc.sync.dma_start(out=xq_sl[:], in_=xq_d[s4])
                        nc.scalar.dma_start(out=xk_sl[:], in_=xk_d[s4])
                        nc.gpsimd.dma_start(out=xv_sl[:], in_=xv_d[s4])

                        for fc in range(2):
                            fsl = slice(fc * 128, (fc + 1) * 128)
                            q_ps = ps1.tile([128, CW], FP32, tag="q_ps")
                            for d in range(PD):
                                nc.tensor.matmul(q_ps[:], wq[:, d, fsl], xq_sl[:, d, :],
                                                 start=(d == 0), stop=(d == PD - 1))
                            nc.scalar.add(qT[:, fc, sl], q_ps[:], bq_sb[:, fc, :])

                            k_ps = ps1.tile([128, CW], FP32, tag="k_ps")
                            for d in range(PD):
                                nc.tensor.matmul(k_ps[:], wk[:, d, fsl], xk_sl[:, d, :],
                                                 start=(d == 0), stop=(d == PD - 1))
                            nc.scalar.add(kT[:, fc, sl], k_ps[:], bk_sb[:, fc, :])

                        for m in range(CW // 128):  # s-subtiles of 128
                            ti = s4 * (CW // 128) + m
                            msl = slice(m * 128, (m + 1) * 128)
                            v_ps = ps1.tile([128, F], FP32, tag="v_ps")
                            for d in range(PD):
                                nc.tensor.matmul(v_ps[:], xv_sl[:, d, msl], wv[:, d, :],
                                                 start=(d == 0), stop=(d == PD - 1))
                            for hh in range(HL):
                                hsl = slice(hh * DH, (hh + 1) * DH)
                                nc.vector.tensor_add(
                                    v2[:, ti, hh, 0:DH],
                                    v_ps[:, hsl],
                                    bv_bc[:, hsl],
                                )

            # ---- phase 2: attention (head pair outer, sq inner) ----
            with tc.tile_pool(name="p2", bufs=2) as p2, \
                 tc.tile_pool(name="p2e", bufs=4) as p2e, \
                 tc.tile_pool(name="rec", bufs=2) as rec, \
                 tc.tile_pool(name="psum_st", bufs=2, space="PSUM") as ps_st, \
                 tc.tile_pool(name="psum_u", bufs=4, space="PSUM") as ps_u:
                wu_ps = ps_st.tile([128, 2, SQC], FP32, tag="st", name="wu_ps")
                for i in range(40):
                    nc.tensor.matmul(wu_ps[0:64, 0, 0:64], wu[:], wu[:],
                                     start=True, stop=True)
                for hp in range(2):
                    for sq in range(NSQ):
                        qsl = slice(sq * SQC, (sq + 1) * SQC)
                        keep = p2.tile([128, SKT, SQC], FP16, tag="keep")
                        nc.gpsimd.dma_start(out=keep[:], in_=keep_d[sq])
                        u = [ps_u.tile([128, 1, SQC], FP32, tag="u", name=f"u_{hp}_{sq}_{j}")
                             for j in range(2)]
                        for sk in range(SKT):
                            ksl = slice(sk * 128, (sk + 1) * 128)
                            st_ps = ps_st.tile([128, 2, SQC], FP32, tag="st")
                            nc.tensor.matmul(st_ps[:, 0, :], kT[0:64, hp, ksl],
                                             qT[0:64, hp, qsl], start=True, stop=True,
                                             tile_position=(0, 0))
                            nc.tensor.matmul(st_ps[:, 1, :], kT[64:128, hp, ksl],
                                             qT[64:128, hp, qsl], start=True, stop=True,
                                             tile_position=(64, 0))
                            e_sb = p2e.tile([128, 2, SQC], FP16, tag="e")
                            nc.scalar.activation(e_sb[:], st_ps[:], Exp, scale=0.125)
                            e2 = p2e.tile([128, 2, SQC], FP16, tag="e2")
                            nc.vector.tensor_mul(e2[:, 0, :], e_sb[:, 0, :], keep[:, sk, :])
                            nc.vector.tensor_mul(e2[:, 1, :], e_sb[:, 1, :], keep[:, sk, :])
                            for j in range(2):
                                nc.tensor.matmul(
                                    u[j][0:DH + 1, 0, :],
                                    v2[:, sk, 2 * hp + j, :],
                                    e2[:, j, :],
                                    start=(sk == 0), stop=(sk == SKT - 1),
                                )
                        for j in range(2):
                            nc.vector.tensor_copy(sums[2 * hp + j][0:1, qsl],
                                                  u[j][DH:DH + 1, 0, :])
                            nc.scalar.copy(ctxT[j * DH:(j + 1) * DH, hp, qsl],
                                           u[j][0:DH, 0, :])
                    # per-pair batched reciprocal r = exp(-ln(sum))
                    lns = [rec.tile([1, S], FP32, tag=f"lns{j}", name=f"lns{hp}_{j}", bufs=1)
                           for j in range(2)]
                    with tc.tile_critical():
                        for j in range(2):
                            nc.scalar.activation(lns[j][:], sums[2 * hp + j][:], Ln)
                        for j in range(2):
                            nc.scalar.activation(sums[2 * hp + j][:], lns[j][:], Exp,
                                                 scale=-1.0)
                    for j in range(2):
                        rb = rec.tile([128, S], FP32, tag="rb")
                        nc.gpsimd.partition_broadcast(rb[:], sums[2 * hp + j][:])
                        nc.vector.tensor_mul(ctxT[j * DH:(j + 1) * DH, hp, :],
                                             ctxT[j * DH:(j + 1) * DH, hp, :],
                                             rb[j * DH:(j + 1) * DH, :])

            # ---- phase 3: output projection ----
            with tc.tile_pool(name="p3", bufs=4) as p3, \
                 tc.tile_pool(name="psum3", bufs=4, space="PSUM") as ps3:
                wu3_ps = ps3.tile([128, 512], FP32, tag="o_ps", name="wu3_ps")
                for i in range(40):
                    nc.tensor.matmul(wu3_ps[0:64, 0:64], wu[:], wu[:],
                                     start=True, stop=True)
                for ti in range(S // 128):
                    tsl = slice(ti * 128, (ti + 1) * 128)
                    for n in range(2):
                        nsl = slice(n * 512, (n + 1) * 512)
                        o_ps = ps3.tile([128, 512], FP32, tag="o_ps")
                        for fc in range(2):
                            nc.tensor.matmul(o_ps[:], ctxT[:, fc, tsl], wo[:, fc, nsl],
                                             start=(fc == 0), stop=(fc == 1))
                        o_sb = p3.tile([128, 512], FP32, tag="o_sb")
                        if (ti * 2 + n) % 2 == 0:
                            nc.vector.tensor_copy(o_sb[:], o_ps[:])
                        else:
                            nc.scalar.copy(o_sb[:], o_ps[:])
                        nc.sync.dma_start(out=out_d[tsl, nsl], in_=o_sb[:])

    nc.compile()
    return nc


def _tile_x(xT):
    # (D, S) -> (NPC, 128, PD, CW); [s4, p, c, j] = xT[c*128+p, s4*CW+j]
    return np.ascontiguousarray(
        xT.reshape(PD, 128, NPC, CW).transpose(2, 1, 0, 3))


def kernel(query, key, value, mask, Wq, bq, Wk, bk, Wv, bv, Wo, bo, **_):
    if "nc" not in _CACHE:
        _CACHE["nc"] = _build()
    nc = _CACHE["nc"]

    query = np.asarray(query, np.float32)
    key = np.asarray(key, np.float32)
    value = np.asarray(value, np.float32)
    mask = np.asarray(mask)
    Wq = np.asarray(Wq, np.float32)
    Wk = np.asarray(Wk, np.float32)
    Wv = np.asarray(Wv, np.float32)
    Wo = np.asarray(Wo, np.float32)
    bq = np.asarray(bq, np.float32)
    bk = np.asarray(bk, np.float32)
    bv = np.asarray(bv, np.float32)
    bo = np.asarray(bo, np.float32)

    xT = {}
    keepT = {}
    for b in range(B):
        xT[b] = (
            _tile_x(query[b].T),
            _tile_x(key[b].T),
            _tile_x(value[b].T),
        )
        kp = (~mask[b]).T.astype(np.float16)  # (sk, sq)
        keepT[b] = np.ascontiguousarray(
            kp.reshape(SKT, 128, NSQ, SQC).transpose(2, 1, 0, 3))

    wsl = {}
    for g in range(GROUPS):
        fs = slice(g * F, (g + 1) * F)
        wsl[g] = (
            np.ascontiguousarray(Wq[:, fs].reshape(PD, 128, F).transpose(1, 0, 2)),
            np.ascontiguousarray(Wk[:, fs].reshape(PD, 128, F).transpose(1, 0, 2)),
            np.ascontiguousarray(Wv[:, fs].reshape(PD, 128, F).transpose(1, 0, 2)),
            np.ascontiguousarray(
                Wo[fs, :].astype(np.float16).reshape(2, 128, D).transpose(1, 0, 2)),
            np.ascontiguousarray(bq[fs].reshape(F, 1)),
            np.ascontiguousarray(bk[fs].reshape(F, 1)),
            np.ascontiguousarray(bv[fs].reshape(1, F)),
        )

    in_maps = []
    for c in range(NCORES):
        b, g = c // GROUPS, c % GROUPS
        wq_s, wk_s, wv_s, wo_s, bq_s, bk_s, bv_s = wsl[g]
        in_maps.append({
            "xqT": xT[b][0], "xkT": xT[b][1], "xvT": xT[b][2],
            "keepT": keepT[b],
            "Wq": wq_s, "Wk": wk_s, "Wv": wv_s, "Wo": wo_s,
            "bq": bq_s, "bk": bk_s, "bv": bv_s,
        })

    res = run_bass_kernel_spmd(nc, in_maps, core_ids=list(range(NCORES)))
    outs = [r["out"] for r in res.results]
    full = np.empty((B, S, D), np.float32)
    for b in range(B):
        acc = outs[GROUPS * b].astype(np.float32)
        for g in range(1, GROUPS):
            acc = acc + outs[GROUPS * b + g]
        full[b] = acc + bo
    return full
